# revision 23
# baseline (speedup 1.0000x reference)
"""Trainium2 Bass kernel: GRU encoder-decoder with Bahdanau attention.

Model: B=4096, T=56 enc steps, S=28 dec steps, H=126.
Sharding: pure data parallel, batch 4096 -> 8 cores x 512.

v2 layout/engine plan:
  - All fp32 gate/proj matmuls run as float32r (single-pass PE, 1 cyc/row
    at N>=256) via AP bitcast; h state stays fp32.
  - GRU biases folded into ACT bias operands; the r*hn product uses
    pre-halved Whh_n plus a K=1 ones-row bias matmul so the hn PSUM is
    consumed directly by one scalar_tensor_tensor.
  - Decoder runs two batch halves (256 each) software-pipelined so the
    DVE/ACT serial chains of one half overlap the other's.
  - Scores: xq = tanh(Uo + Wh) as one DVE add + one ACT tanh per half;
    V-dot via M=1 matmuls (N=448, col-group packed 4x) -> PSUM -> direct
    DMA into batch-partitioned scf.
  - Softmax drops the max-subtraction (|score| <= ||V||_1 ~ 8, safe in
    fp32) and V_b (shift-invariant); 1/sum folded into alpha.
  - Attention sum: one TT mult + one bf16 tensor_reduce per half.
"""
import sys
import numpy as np

for _p in ('/opt/trn_rl_repo', '/root/.axon_site/_ro/trn_rl_repo'):
    if _p not in sys.path:
        sys.path.insert(0, _p)

from concourse import bass, tile
from concourse.vector_clock import ScopedClock

mybir = bass.mybir
F32 = mybir.dt.float32
F32R = mybir.dt.float32r
BF16 = mybir.dt.bfloat16
AF = mybir.ActivationFunctionType
ALU = mybir.AluOpType
AX = mybir.AxisListType

# ---- workaround: this walrus build allows only one embedded sync-wait on
# the Tile tail drain; spread the global-clock waits over SP nops instead.
def _patched_drain_and_barrier(self, tick_clock, wait_clock):
    nc = self.nc
    probe = nc.sync.nop()
    wait_clock.add_sem_waits(probe.ins, ScopedClock({None: tick_clock.global_clock}))
    si = probe.ins.sync_info
    waits = list(si.on_wait or []) if si is not None else []
    if si is not None:
        si.on_wait = waits[:1]
    for w in waits[1:]:
        n2 = nc.sync.nop()
        n2.ins.sync_info = mybir.SyncInfo(on_wait=[w], on_update=[])
    nc.sync.drain()
    nc.all_engine_barrier()
    popped = nc._tile_sem_poison_stack.pop()
    assert popped is self._sem_poison
    nc.clear_and_free_semaphores(list(self.sems.allocated().values()))
    nc.all_engine_barrier()

tile.TileContext._drain_and_barrier = _patched_drain_and_barrier


def _split_excess_waits(nc):
    """This walrus build allows 1 embedded sync-wait per instruction; move
    extras onto same-engine nops inserted just before the instruction."""
    cnt = 0
    for _, bassbb in list(nc.bb_map.items()):
        bb = bassbb.bb if hasattr(bassbb, "bb") else bassbb
        il = bb.instructions
        i = 0
        while i < len(il):
            inst = il[i]
            si = inst.sync_info
            if si is not None and si.on_wait and len(si.on_wait) > 1:
                extra = list(si.on_wait[:-1])
                si.on_wait = [si.on_wait[-1]]
                for w in extra:
                    cnt += 1
                    nop = mybir.InstNoOp(name=f"wfix-{cnt}", ins=[], outs=[])
                    nop.engine = inst.engine
                    nop.sync_info = mybir.SyncInfo(on_wait=[w], on_update=[])
                    il.insert(i, nop)
                    i += 1
            i += 1
    return cnt

B, T, S = 4096, 56, 28
H, ANN, ENC, DEC = 126, 30, 20, 15
NCORES = 8
BS = B // NCORES          # 512 batch per core
NH = 2                    # decoder batch halves (software pipeline)
HB = BS // NH             # 256 batch per half
NC2 = HB // 128           # 2 chunks of 128 per half
TQ = 14                   # t-block for the V-dot matmuls
BBLK = 32                 # batch block per V-dot matmul

_CACHE = {}


def _build_program():
    import os
    kt = int(os.environ.get("K_T", T))
    ks = int(os.environ.get("K_S", S))
    nc = bass.Bass()

    di = lambda name, shape, dt=F32: nc.declare_dram_parameter(name, list(shape), dt, isOutput=False)
    enc_d = di("enc", (T, ENC, BS), BF16)
    dec_d = di("dec", (S, DEC, BS), BF16)
    ann_d = di("ann", (ANN, BS), BF16)
    W1_d = di("W1", (ANN, 96), BF16)
    W2_d = di("W2", (96, H), BF16)
    b1_d = di("b1", (96, 1))
    Wih_e_d = di("Wih_e", (ENC, 3 * H), BF16)
    Whh_e_d = di("Whh_e", (H, 3 * H), BF16)
    WihP_d_d = di("WihP_d", (1, 3 * H), BF16)
    WihX_d_d = di("WihX_d", (DEC, 3 * H), BF16)
    WihA_d_d = di("WihA_d", (H, 3 * H), BF16)
    Whh_d_d = di("Whh_d", (H, 3 * H), BF16)
    UW_d = di("UW", (H, H), BF16)
    WlW_d = di("WlW", (H, H), BF16)
    B_d = di("BIAS", (H, 16))
    BR_d = di("BR", (1, 2 * H), BF16)
    h2ob_d = di("h2ob", (1, 1))
    id_d = di("ident", (128, 128))
    out_d = nc.declare_dram_parameter("out", [S, BS], BF16, isOutput=True)

    from contextlib import ExitStack
    with tile.TileContext(nc) as tc, ExitStack() as es:
        cp = es.enter_context(tc.tile_pool(name="const", bufs=1))
        sp = es.enter_context(tc.tile_pool(name="sb", bufs=2))
        hp = es.enter_context(tc.tile_pool(name="hs", bufs=4))
        xqp = es.enter_context(tc.tile_pool(name="xq", bufs=3))
        pbp = es.enter_context(tc.tile_pool(name="pb", bufs=2))
        ppg = es.enter_context(tc.tile_pool(name="psg", bufs=2, space="PSUM"))
        ppn = es.enter_context(tc.tile_pool(name="psn", bufs=1, space="PSUM"))
        ppw = es.enter_context(tc.tile_pool(name="psw", bufs=1, space="PSUM"))
        ppsc = es.enter_context(tc.tile_pool(name="pssc", bufs=2, space="PSUM"))
        pptr = es.enter_context(tc.tile_pool(name="pstr", bufs=1, space="PSUM"))
        gp = es.enter_context(tc.tile_pool(name="gates", bufs=8))
        mp = es.enter_context(tc.tile_pool(name="misc", bufs=2))

        def cload(dram, shape, dtype=F32):
            t_ = cp.tile(list(shape), dtype, tag="c_" + dram.name)
            nc.sync.dma_start(out=t_[:], in_=dram[:])
            return t_

        W1 = cload(W1_d, (ANN, 96), BF16)
        W2 = cload(W2_d, (96, H), BF16)
        b1 = cload(b1_d, (96, 1))
        Wih_e = cload(Wih_e_d, (ENC, 3 * H), BF16)
        Whh_e = cload(Whh_e_d, (H, 3 * H), BF16)
        WihP = cload(WihP_d_d, (1, 3 * H), BF16)
        WihX = cload(WihX_d_d, (DEC, 3 * H), BF16)
        WihA = cload(WihA_d_d, (H, 3 * H), BF16)
        Whh_dd = cload(Whh_d_d, (H, 3 * H), BF16)
        UW = cload(UW_d, (H, H), BF16)
        WlW = cload(WlW_d, (H, H), BF16)
        BI = cload(B_d, (H, 16))
        BR = cload(BR_d, (1, 2 * H), BF16)
        h2ob = cload(h2ob_d, (1, 1))
        idf = cload(id_d, (128, 128))
        idb = cp.tile([128, 128], BF16, tag="idb")
        nc.vector.tensor_copy(idb[:], idf[:])
        Vb = cp.tile([H, 1], BF16, tag="Vb")
        nc.vector.tensor_copy(Vb[:], BI[:, 10:11])
        h2oWb = cp.tile([H, 1], BF16, tag="h2oWb")
        nc.vector.tensor_copy(h2oWb[:], BI[:, 11:12])
        ones = cp.tile([1, BS], BF16, tag="ones")
        nc.vector.memset(ones[:], 1.0)

        # persistent big tensors
        Uo = cp.tile([H, T, BS], BF16, tag="Uo")            # 57.3 KB/part
        encb = cp.tile([128, NH * NC2, H, T], BF16, tag="encb")  # 56.4 KB/part

        # bias columns (r/z biases pre-halved on host)
        bre, bze, bine = BI[:, 0:1], BI[:, 1:2], BI[:, 2:3]
        brd, bzd, bind = BI[:, 4:5], BI[:, 5:6], BI[:, 6:7]
        Ub, Wlb, h2oW, b2 = BI[:, 8:9], BI[:, 9:10], BI[:, 11:12], BI[:, 12:13]

        # ---------------- phase A: static -> h0 ----------------
        annt = sp.tile([ANN, BS], BF16, tag="x")
        nc.sync.dma_start(out=annt[:], in_=ann_d[:])
        ps96 = ppg.tile([96, BS], F32, tag="grz")
        nc.tensor.matmul(ps96[:], W1[:], annt[:], start=True, stop=True)
        hid1 = sp.tile([96, BS], BF16, tag="hid")
        nc.scalar.activation(hid1[:], ps96[:], AF.Relu, bias=b1[:, 0:1])
        psh = ppg.tile([H, BS], F32, tag="grz")
        nc.tensor.matmul(psh[:], W2[:], hid1[:], start=True, stop=True)
        hbb = [hp.tile([H, HB], BF16, tag=f"hb{hf}", name=f"hb{hf}") for hf in range(NH)]
        for hf in range(NH):
            nc.scalar.activation(hbb[hf][:], psh[:, hf * HB:(hf + 1) * HB],
                                 AF.Identity, bias=b2)

        # one GRU tail, consumes gate psums -> h_new (per half)
        # ps_rz: [H,2,HB] (r,z); ps_nh: [H,2,HB] (ni, hn_half incl bias)
        def gru_tail(ps_rz, ps_nh, br_, bz_, bin_, hb_old, hf):
            th_r = gp.tile([H, HB], F32, tag="gate")
            nc.scalar.activation(th_r[:], ps_rz[:, 0, :], AF.Tanh, bias=br_, scale=0.5)
            th_z = gp.tile([H, HB], BF16, tag="gatez")
            nc.scalar.activation(th_z[:], ps_rz[:, 1, :], AF.Tanh, bias=bz_, scale=0.5)
            tmp = gp.tile([H, HB], F32, tag="gate")
            nc.vector.scalar_tensor_tensor(tmp[:], th_r[:], 1.0, ps_nh[:, 1, :],
                                           ALU.add, ALU.mult)
            pre = gp.tile([H, HB], F32, tag="gate")
            nc.vector.tensor_add(pre[:], tmp[:], ps_nh[:, 0, :])
            n_ = gp.tile([H, HB], BF16, tag="gatez")
            nc.scalar.activation(n_[:], pre[:], AF.Tanh, bias=bin_)
            d_ = gp.tile([H, HB], BF16, tag="gatez")
            nc.vector.tensor_sub(d_[:], n_[:], hb_old[:])
            v1 = gp.tile([H, HB], BF16, tag="gatez")
            nc.vector.scalar_tensor_tensor(v1[:], th_z[:], -1.0, d_[:], ALU.add, ALU.mult)
            hb_new = hp.tile([H, HB], BF16, tag=f"hb{hf}", name=f"hbn{hf}")
            nc.vector.scalar_tensor_tensor(hb_new[:], v1[:], -0.5, hb_old[:],
                                           ALU.mult, ALU.add)
            return hb_new

        # ---------------- phase B: encoder ----------------
        for t in range(kt):
            xt = sp.tile([ENC, BS], BF16, tag="x")
            nc.sync.dma_start(out=xt[:], in_=enc_d[t])
            for hf in range(NH):
                sl = slice(hf * HB, (hf + 1) * HB)
                hb_old = hbb[hf]
                ps_rz = ppg.tile([H, 2, HB], F32, tag="grz")
                nc.tensor.matmul(ps_rz[:, 0, :], Wih_e[:, 0:H], xt[:, sl],
                                 start=True, stop=False)
                nc.tensor.matmul(ps_rz[:, 0, :], Whh_e[:, 0:H], hb_old[:],
                                 start=False, stop=True)
                nc.tensor.matmul(ps_rz[:, 1, :], Wih_e[:, H:2 * H], xt[:, sl],
                                 start=True, stop=False)
                nc.tensor.matmul(ps_rz[:, 1, :], Whh_e[:, H:2 * H], hb_old[:],
                                 start=False, stop=True)
                ps_nh = ppn.tile([H, 2, HB], F32, tag="gnh")
                nc.tensor.matmul(ps_nh[:, 0, :], Wih_e[:, 2 * H:3 * H], xt[:, sl],
                                 start=True, stop=True)
                nc.tensor.matmul(ps_nh[:, 1, :], Whh_e[:, 2 * H:3 * H], hb_old[:],
                                 start=True, stop=False)
                nc.tensor.matmul(ps_nh[:, 1, :], BR[0:1, 0:H], ones[:, sl],
                                 start=False, stop=True)
                hb_new = gru_tail(ps_rz, ps_nh, bre, bze, bine, hb_old, hf)

                # Uo[:, t, sl] = U @ h_new + Ub   (bf16)
                ps_uo = ppw.tile([H, HB], F32, tag="wh")
                nc.tensor.matmul(ps_uo[:], UW[:], hb_new[:], start=True, stop=True)
                nc.scalar.activation(Uo[:, t, sl], ps_uo[:], AF.Identity, bias=Ub)

                # encb[:, 2hf+c2, :, t] = h_new.T chunks (bf16)
                for c2 in range(NC2):
                    ptr = pptr.tile([128, 128], BF16, tag="trb")
                    nc.tensor.transpose(ptr[0:128, 0:H],
                                        hb_new[:, c2 * 128:(c2 + 1) * 128],
                                        idb[0:H, 0:H])
                    nc.scalar.copy(encb[:, NC2 * hf + c2, :, t], ptr[0:128, 0:H])
                hbb[hf] = hb_new

        # ---------------- phase C: decoder ----------------
        prevs = []
        for hf in range(NH):
            pv = hp.tile([1, HB], BF16, tag=f"pv{hf}")
            nc.sync.dma_start(out=pv[:], in_=enc_d[T - 1, 0:1, hf * HB:(hf + 1) * HB])
            prevs.append(pv)

        # wh for step 0 (subsequent steps hoist this next to the tail)
        whs = {}

        def compute_wh(hf):
            ps_wh = ppw.tile([H, HB], F32, tag="wh")
            nc.tensor.matmul(ps_wh[:], WlW[:], hbb[hf][:], start=True, stop=True)
            wh = mp.tile([H, 1, HB], BF16, tag="whb")
            nc.scalar.activation(wh[:, 0, :], ps_wh[:], AF.Identity, bias=Wlb)
            whs[hf] = wh

        for hf in range(NH):
            compute_wh(hf)

        for s in range(ks):
            dxt = sp.tile([DEC, BS], BF16, tag="dx")
            nc.sync.dma_start(out=dxt[:], in_=dec_d[s])

            scfs, attns = {}, {}
            # scores pipeline: interleave the halves' (add, tanh) chunks, with
            # each chunk's V-dot matmuls + staging copies emitted one slot
            # behind so copies never head the ACT queue ahead of a tanh
            for hf in range(NH):
                scfs[hf] = sp.tile([128, NC2, T], F32, tag=f"sco{hf}",
                                   name=f"sco{hf}")
            seq = [(hf, q) for hf in range(NH) for q in range(T // TQ)]
            xqs = {}

            def vdot_group(hf, q):
                xq = xqs[(hf, q)]
                tq = slice(q * TQ, (q + 1) * TQ)
                for c2 in range(NC2):
                    pssc = ppsc.tile([128, TQ, BBLK], F32, tag="sc")
                    for j in range(4):
                        b0 = c2 * 128 + j * BBLK
                        nc.tensor.matmul(pssc[BBLK * j:BBLK * j + 1, :, :], Vb[:],
                                         xq[:, :, b0:b0 + BBLK],
                                         start=True, stop=True,
                                         tile_position=(0, BBLK * j))
                    sstg = mp.tile([128, BBLK, TQ], F32, tag="sst")
                    nc.scalar.copy(sstg[:], pssc[:].transpose([0, 2, 1]))
                    nc.sync.dma_start(out=scfs[hf][:, c2, tq],
                                      in_=sstg[0:128:BBLK])

            for hf in range(NH):
                sl = slice(hf * HB, (hf + 1) * HB)
                for q in range(T // TQ):
                    tq = slice(q * TQ, (q + 1) * TQ)
                    xq = xqp.tile([H, TQ, HB], BF16, tag="xt")
                    nc.vector.tensor_add(xq[:], Uo[:, tq, sl],
                                         whs[hf][:].broadcast_to((H, TQ, HB)))
                    nc.scalar.activation(xq[:], xq[:], AF.Tanh)
                    xqs[(hf, q)] = xq
                    if q > 0:
                        vdot_group(hf, q - 1)
                vdot_group(hf, T // TQ - 1)

            # stage softmax (no max-subtraction; scores bounded by ||V||_1)
            # + attention weighted sum + transpose back
            for hf in range(NH):
                scf = scfs[hf]
                expo = sp.tile([128, NC2, T], F32, tag="expo")
                nc.scalar.activation(expo[:], scf[:], AF.Exp)
                sm = sp.tile([128, NC2], F32, tag="red")
                nc.vector.tensor_reduce(sm[:], expo[:], axis=AX.X, op=ALU.add)
                inv = sp.tile([128, NC2], F32, tag="red2")
                nc.vector.reciprocal(inv[:], sm[:])
                ab = sp.tile([128, NC2, 1, T], BF16, tag="ab")
                for c2 in range(NC2):
                    nc.vector.tensor_scalar_mul(ab[:, c2, 0, :], expo[:, c2, :],
                                                inv[:, c2:c2 + 1])
                attn_h = mp.tile([H, HB], BF16, tag="ah")
                TH = T // 2
                for c2 in range(NC2):
                    # alpha-weighted products, then in-place bf16 fold tree
                    # (TT adds run 2x; TensorReduce only 1x) down to 7 t-slots
                    P1 = pbp.tile([128, H, TH], BF16, tag="P")
                    nc.vector.tensor_mul(
                        P1[:], encb[:, NC2 * hf + c2, :, 0:TH],
                        ab[:, c2, :, 0:TH].broadcast_to((128, H, TH)))
                    P2 = pbp.tile([128, H, TH], BF16, tag="P")
                    nc.vector.tensor_mul(
                        P2[:], encb[:, NC2 * hf + c2, :, TH:T],
                        ab[:, c2, :, TH:T].broadcast_to((128, H, TH)))
                    nc.vector.tensor_add(P1[:], P1[:], P2[:])
                    nc.vector.tensor_add(P1[:, :, 0:14], P1[:, :, 0:14], P1[:, :, 14:28])
                    nc.vector.tensor_add(P1[:, :, 0:7], P1[:, :, 0:7], P1[:, :, 7:14])
                    attnc = sp.tile([128, H], BF16, tag="attnc")
                    with nc.allow_low_precision(reason="bf16 attn t-reduce"):
                        nc.vector.tensor_reduce(attnc[:], P1[:, :, 0:7], axis=AX.X,
                                                op=ALU.add)
                    ptr = pptr.tile([128, 128], BF16, tag="trb")
                    nc.tensor.transpose(ptr[0:H, 0:128], attnc[:], idb[:])
                    nc.scalar.copy(attn_h[:, c2 * 128:(c2 + 1) * 128], ptr[0:H, 0:128])
                attns[hf] = attn_h

            # stage gates + tail + out per half
            for hf in range(NH):
                sl = slice(hf * HB, (hf + 1) * HB)
                hb_old = hbb[hf]
                attn_h = attns[hf]
                prev = prevs[hf]
                ps_rz = ppg.tile([H, 2, HB], F32, tag="grz")
                for gi, g0 in ((0, 0), (1, H)):
                    nc.tensor.matmul(ps_rz[:, gi, :], WihP[:, g0:g0 + H], prev[:],
                                     start=True, stop=False)
                    nc.tensor.matmul(ps_rz[:, gi, :], WihX[:, g0:g0 + H], dxt[:, sl],
                                     start=False, stop=False)
                    nc.tensor.matmul(ps_rz[:, gi, :], WihA[:, g0:g0 + H], attn_h[:],
                                     start=False, stop=False)
                    nc.tensor.matmul(ps_rz[:, gi, :], Whh_dd[:, g0:g0 + H], hb_old[:],
                                     start=False, stop=True)
                ps_nh = ppn.tile([H, 2, HB], F32, tag="gnh")
                g0 = 2 * H
                nc.tensor.matmul(ps_nh[:, 0, :], WihP[:, g0:g0 + H], prev[:],
                                 start=True, stop=False)
                nc.tensor.matmul(ps_nh[:, 0, :], WihX[:, g0:g0 + H], dxt[:, sl],
                                 start=False, stop=False)
                nc.tensor.matmul(ps_nh[:, 0, :], WihA[:, g0:g0 + H], attn_h[:],
                                 start=False, stop=True)
                nc.tensor.matmul(ps_nh[:, 1, :], Whh_dd[:, g0:g0 + H], hb_old[:],
                                 start=True, stop=False)
                nc.tensor.matmul(ps_nh[:, 1, :], BR[0:1, H:2 * H], ones[:, sl],
                                 start=False, stop=True)
                hb_new = gru_tail(ps_rz, ps_nh, brd, bzd, bind, hb_old, hf)

                # out_s = h2o @ h_new + b  -> DRAM, also feeds prev
                ps_o = pptr.tile([1, HB], F32, tag="osc")
                nc.tensor.matmul(ps_o[:], h2oWb[:], hb_new[:], start=True, stop=True)
                pv = hp.tile([1, HB], BF16, tag=f"pv{hf}")
                nc.scalar.activation(pv[:], ps_o[:], AF.Identity, bias=h2ob[:, 0:1])
                nc.sync.dma_start(out=out_d[s, sl], in_=pv[:])
                prevs[hf] = pv
                hbb[hf] = hb_new
                if s < ks - 1:
                    compute_wh(hf)
    _split_excess_waits(nc)
    return nc


def _host_inputs(inputs):
    import ml_dtypes
    f = lambda a: np.ascontiguousarray(a, dtype=np.float32)
    g = lambda a: np.ascontiguousarray(np.asarray(a, dtype=np.float32),
                                       dtype=ml_dtypes.bfloat16)
    Whh_e = np.asarray(inputs["enc_Whh"]).T.copy()   # [H, 3H]
    Whh_d = np.asarray(inputs["dec_Whh"]).T.copy()
    Whh_e[:, 2 * H:3 * H] *= 0.5
    Whh_d[:, 2 * H:3 * H] *= 0.5
    shared = {
        "W1": g(inputs["s2h_W1"].T), "W2": g(inputs["s2h_W2"].T),
        "b1": f(np.asarray(inputs["s2h_b1"]).reshape(96, 1)),
        "Wih_e": g(inputs["enc_Wih"].T), "Whh_e": g(Whh_e),
        "WihP_d": g(inputs["dec_Wih"][:, 0:1].T),
        "WihX_d": g(inputs["dec_Wih"][:, 1:1 + DEC].T),
        "WihA_d": g(inputs["dec_Wih"][:, 1 + DEC:].T),
        "Whh_d": g(Whh_d),
        "UW": g(inputs["U_W"].T), "WlW": g(inputs["Wl_W"].T),
        "h2ob": f(np.asarray(inputs["h2o_b"]).reshape(1, 1)),
        "ident": f(np.eye(128)),
    }
    BI = np.zeros((H, 16), dtype=np.float32)
    ebih, ebhh = np.asarray(inputs["enc_bih"]), np.asarray(inputs["enc_bhh"])
    dbih, dbhh = np.asarray(inputs["dec_bih"]), np.asarray(inputs["dec_bhh"])
    BI[:, 0] = (ebih[0:H] + ebhh[0:H]) / 2
    BI[:, 1] = (ebih[H:2 * H] + ebhh[H:2 * H]) / 2
    BI[:, 2] = ebih[2 * H:3 * H]
    BI[:, 4] = (dbih[0:H] + dbhh[0:H]) / 2
    BI[:, 5] = (dbih[H:2 * H] + dbhh[H:2 * H]) / 2
    BI[:, 6] = dbih[2 * H:3 * H]
    BI[:, 8] = np.asarray(inputs["U_b"])
    BI[:, 9] = np.asarray(inputs["Wl_b"])
    BI[:, 10] = np.asarray(inputs["V_W"]).reshape(-1)
    BI[:, 11] = np.asarray(inputs["h2o_W"]).reshape(-1)
    BI[:, 12] = np.asarray(inputs["s2h_b2"])
    shared["BIAS"] = f(BI)
    BR = np.zeros((1, 2 * H), dtype=np.float32)
    BR[0, 0:H] = ebhh[2 * H:3 * H] / 2
    BR[0, H:2 * H] = dbhh[2 * H:3 * H] / 2
    shared["BR"] = g(BR)

    enc = np.asarray(inputs["encoder_data"])   # [T, B, ENC]
    dec = np.asarray(inputs["decoder_data"])   # [S, B, DEC]
    ann = np.asarray(inputs["ann_data"])       # [B, ANN]
    maps = []
    for i in range(NCORES):
        sl = slice(i * BS, (i + 1) * BS)
        m = dict(shared)
        m["enc"] = g(enc[:, sl, :].transpose(0, 2, 1))
        m["dec"] = g(dec[:, sl, :].transpose(0, 2, 1))
        m["ann"] = g(ann[sl, :].T)
        maps.append(m)
    return maps


def kernel(**inputs) -> np.ndarray:
    from concourse.bass_utils import run_bass_kernel_spmd
    if "nc" not in _CACHE:
        _CACHE["nc"] = _build_program()
    nc = _CACHE["nc"]
    maps = _host_inputs(inputs)
    import os
    kw = {}
    if os.environ.get("KERNEL_TRACE") == "1":
        kw = dict(trace=True, trace_cores=[0])
    res = run_bass_kernel_spmd(nc, maps, list(range(NCORES)), **kw)
    _CACHE["last_res"] = res
    outs = [np.asarray(res.results[i]["out"], dtype=np.float32) for i in range(NCORES)]
    full = np.stack(outs, axis=1)              # [S, 8, 512]
    return full.reshape(S, B, 1)


if __name__ == "__main__":
    rng = np.random.default_rng(0)
    fake = {}
    fake["ann_data"] = rng.standard_normal((B, ANN), dtype=np.float32)
    fake["encoder_data"] = rng.standard_normal((T, B, ENC), dtype=np.float32)
    fake["decoder_data"] = rng.standard_normal((S, B, DEC), dtype=np.float32)
    p = lambda *s: (rng.standard_normal(s, dtype=np.float32) * 0.08)
    fake.update({
        "s2h_W1": p(96, ANN), "s2h_b1": p(96), "s2h_W2": p(H, 96), "s2h_b2": p(H),
        "enc_Wih": p(3 * H, ENC), "enc_Whh": p(3 * H, H),
        "enc_bih": p(3 * H), "enc_bhh": p(3 * H),
        "dec_Wih": p(3 * H, 1 + DEC + H), "dec_Whh": p(3 * H, H),
        "dec_bih": p(3 * H), "dec_bhh": p(3 * H),
        "U_W": p(H, H), "U_b": p(H), "Wl_W": p(H, H), "Wl_b": p(H),
        "V_W": p(1, H), "V_b": p(1), "h2o_W": p(1, H), "h2o_b": p(1),
    })
    out = kernel(**fake)
    print("out", out.shape, out.dtype, float(np.abs(out).mean()))


# revision 24
# speedup vs baseline: 1.0551x; 1.0551x over previous
"""Trainium2 Bass kernel: GRU encoder-decoder with Bahdanau attention.

Model: B=4096, T=56 enc steps, S=28 dec steps, H=126.
Sharding: pure data parallel, batch 4096 -> 8 cores x 512.

v2 layout/engine plan:
  - All fp32 gate/proj matmuls run as float32r (single-pass PE, 1 cyc/row
    at N>=256) via AP bitcast; h state stays fp32.
  - GRU biases folded into ACT bias operands; the r*hn product uses
    pre-halved Whh_n plus a K=1 ones-row bias matmul so the hn PSUM is
    consumed directly by one scalar_tensor_tensor.
  - Decoder runs two batch halves (256 each) software-pipelined so the
    DVE/ACT serial chains of one half overlap the other's.
  - Scores: xq = tanh(Uo + Wh) as one DVE add + one ACT tanh per half;
    V-dot via M=1 matmuls (N=448, col-group packed 4x) -> PSUM -> direct
    DMA into batch-partitioned scf.
  - Softmax drops the max-subtraction (|score| <= ||V||_1 ~ 8, safe in
    fp32) and V_b (shift-invariant); 1/sum folded into alpha.
  - Attention sum: one TT mult + one bf16 tensor_reduce per half.
"""
import sys
import numpy as np

for _p in ('/opt/trn_rl_repo', '/root/.axon_site/_ro/trn_rl_repo'):
    if _p not in sys.path:
        sys.path.insert(0, _p)

from concourse import bass, tile
from concourse.vector_clock import ScopedClock

mybir = bass.mybir
F32 = mybir.dt.float32
F32R = mybir.dt.float32r
BF16 = mybir.dt.bfloat16
AF = mybir.ActivationFunctionType
ALU = mybir.AluOpType
AX = mybir.AxisListType

# ---- workaround: this walrus build allows only one embedded sync-wait on
# the Tile tail drain; spread the global-clock waits over SP nops instead.
def _patched_drain_and_barrier(self, tick_clock, wait_clock):
    nc = self.nc
    probe = nc.sync.nop()
    wait_clock.add_sem_waits(probe.ins, ScopedClock({None: tick_clock.global_clock}))
    si = probe.ins.sync_info
    waits = list(si.on_wait or []) if si is not None else []
    if si is not None:
        si.on_wait = waits[:1]
    for w in waits[1:]:
        n2 = nc.sync.nop()
        n2.ins.sync_info = mybir.SyncInfo(on_wait=[w], on_update=[])
    nc.sync.drain()
    nc.all_engine_barrier()
    popped = nc._tile_sem_poison_stack.pop()
    assert popped is self._sem_poison
    nc.clear_and_free_semaphores(list(self.sems.allocated().values()))
    nc.all_engine_barrier()

tile.TileContext._drain_and_barrier = _patched_drain_and_barrier


def _split_excess_waits(nc):
    """This walrus build allows 1 embedded sync-wait per instruction; move
    extras onto same-engine nops inserted just before the instruction."""
    cnt = 0
    for _, bassbb in list(nc.bb_map.items()):
        bb = bassbb.bb if hasattr(bassbb, "bb") else bassbb
        il = bb.instructions
        i = 0
        while i < len(il):
            inst = il[i]
            si = inst.sync_info
            if si is not None and si.on_wait and len(si.on_wait) > 1:
                extra = list(si.on_wait[:-1])
                si.on_wait = [si.on_wait[-1]]
                for w in extra:
                    cnt += 1
                    nop = mybir.InstNoOp(name=f"wfix-{cnt}", ins=[], outs=[])
                    nop.engine = inst.engine
                    nop.sync_info = mybir.SyncInfo(on_wait=[w], on_update=[])
                    il.insert(i, nop)
                    i += 1
            i += 1
    return cnt

B, T, S = 4096, 56, 28
H, ANN, ENC, DEC = 126, 30, 20, 15
NCORES = 8
BS = B // NCORES          # 512 batch per core
NH = 2                    # decoder batch halves (software pipeline)
HB = BS // NH             # 256 batch per half
NC2 = HB // 128           # 2 chunks of 128 per half
TQ = 14                   # t-block for the V-dot matmuls
BBLK = 32                 # batch block per V-dot matmul

_CACHE = {}


def _build_program():
    import os
    kt = int(os.environ.get("K_T", T))
    ks = int(os.environ.get("K_S", S))
    nc = bass.Bass()

    di = lambda name, shape, dt=F32: nc.declare_dram_parameter(name, list(shape), dt, isOutput=False)
    enc_d = di("enc", (T, ENC, BS), BF16)
    dec_d = di("dec", (S, DEC, BS), BF16)
    ann_d = di("ann", (ANN, BS), BF16)
    W1_d = di("W1", (ANN, 96), BF16)
    W2_d = di("W2", (96, H), BF16)
    b1_d = di("b1", (96, 1))
    Wih_e_d = di("Wih_e", (ENC, 3 * H), BF16)
    Whh_e_d = di("Whh_e", (H, 3 * H), BF16)
    WihP_d_d = di("WihP_d", (1, 3 * H), BF16)
    WihX_d_d = di("WihX_d", (DEC, 3 * H), BF16)
    WihA_d_d = di("WihA_d", (H, 3 * H), BF16)
    Whh_d_d = di("Whh_d", (H, 3 * H), BF16)
    UW_d = di("UW", (H, H), BF16)
    WlW_d = di("WlW", (H, H), BF16)
    B_d = di("BIAS", (H, 16))
    BR_d = di("BR", (1, 2 * H), BF16)
    h2ob_d = di("h2ob", (1, 1))
    id_d = di("ident", (128, 128))
    out_d = nc.declare_dram_parameter("out", [S, BS], BF16, isOutput=True)

    from contextlib import ExitStack
    with tile.TileContext(nc) as tc, ExitStack() as es:
        cp = es.enter_context(tc.tile_pool(name="const", bufs=1))
        sp = es.enter_context(tc.tile_pool(name="sb", bufs=2))
        hp = es.enter_context(tc.tile_pool(name="hs", bufs=4))
        xqp = es.enter_context(tc.tile_pool(name="xq", bufs=2))
        pbp = es.enter_context(tc.tile_pool(name="pb", bufs=2))
        ppg = es.enter_context(tc.tile_pool(name="psg", bufs=2, space="PSUM"))
        ppw = es.enter_context(tc.tile_pool(name="psw", bufs=1, space="PSUM"))
        ppsc = es.enter_context(tc.tile_pool(name="pssc", bufs=1, space="PSUM"))
        pptr = es.enter_context(tc.tile_pool(name="pstr", bufs=1, space="PSUM"))
        gp = es.enter_context(tc.tile_pool(name="gates", bufs=8))
        mp = es.enter_context(tc.tile_pool(name="misc", bufs=2))

        def cload(dram, shape, dtype=F32):
            t_ = cp.tile(list(shape), dtype, tag="c_" + dram.name)
            nc.sync.dma_start(out=t_[:], in_=dram[:])
            return t_

        W1 = cload(W1_d, (ANN, 96), BF16)
        W2 = cload(W2_d, (96, H), BF16)
        b1 = cload(b1_d, (96, 1))
        Wih_e = cload(Wih_e_d, (ENC, 3 * H), BF16)
        Whh_e = cload(Whh_e_d, (H, 3 * H), BF16)
        WihP = cload(WihP_d_d, (1, 3 * H), BF16)
        WihX = cload(WihX_d_d, (DEC, 3 * H), BF16)
        WihA = cload(WihA_d_d, (H, 3 * H), BF16)
        Whh_dd = cload(Whh_d_d, (H, 3 * H), BF16)
        UW = cload(UW_d, (H, H), BF16)
        WlW = cload(WlW_d, (H, H), BF16)
        BI = cload(B_d, (H, 16))
        BR = cload(BR_d, (1, 2 * H), BF16)
        h2ob = cload(h2ob_d, (1, 1))
        idf = cload(id_d, (128, 128))
        idb = cp.tile([128, 128], BF16, tag="idb")
        nc.vector.tensor_copy(idb[:], idf[:])
        Vb = cp.tile([H, 1], BF16, tag="Vb")
        nc.vector.tensor_copy(Vb[:], BI[:, 10:11])
        h2oWb = cp.tile([H, 1], BF16, tag="h2oWb")
        nc.vector.tensor_copy(h2oWb[:], BI[:, 11:12])
        ones = cp.tile([1, BS], BF16, tag="ones")
        nc.vector.memset(ones[:], 1.0)

        # persistent big tensors
        Uo = cp.tile([H, T, BS], BF16, tag="Uo")            # 57.3 KB/part
        encb = cp.tile([128, NH * NC2, H, T], BF16, tag="encb")  # 56.4 KB/part

        # bias columns (r/z biases pre-halved on host)
        bre, bze, bine = BI[:, 0:1], BI[:, 1:2], BI[:, 2:3]
        brd, bzd, bind = BI[:, 4:5], BI[:, 5:6], BI[:, 6:7]
        Ub, Wlb, h2oW, b2 = BI[:, 8:9], BI[:, 9:10], BI[:, 11:12], BI[:, 12:13]

        # ---------------- phase A: static -> h0 ----------------
        annt = sp.tile([ANN, BS], BF16, tag="x")
        nc.sync.dma_start(out=annt[:], in_=ann_d[:])
        ps96 = ppg.tile([96, BS], F32, tag="grz")
        nc.tensor.matmul(ps96[:], W1[:], annt[:], start=True, stop=True)
        hid1 = sp.tile([96, BS], BF16, tag="hid")
        nc.scalar.activation(hid1[:], ps96[:], AF.Relu, bias=b1[:, 0:1])
        psh = ppg.tile([H, BS], F32, tag="grz")
        nc.tensor.matmul(psh[:], W2[:], hid1[:], start=True, stop=True)
        hbb = [hp.tile([H, HB], BF16, tag=f"hb{hf}", name=f"hb{hf}") for hf in range(NH)]
        for hf in range(NH):
            nc.scalar.activation(hbb[hf][:], psh[:, hf * HB:(hf + 1) * HB],
                                 AF.Identity, bias=b2)

        # one GRU tail, consumes gate psums -> h_new (per half)
        # ps_rz: [H,2,HB] (r,z); ps_nh: [H,2,HB] (ni, hn_half incl bias)
        def gru_tail(ps_rz, ps_nh, br_, bz_, bin_, hb_old, hf):
            th_r = gp.tile([H, HB], F32, tag="gate")
            nc.scalar.activation(th_r[:], ps_rz[:, 0, :], AF.Tanh, bias=br_, scale=0.5)
            th_z = gp.tile([H, HB], BF16, tag="gatez")
            nc.scalar.activation(th_z[:], ps_rz[:, 1, :], AF.Tanh, bias=bz_, scale=0.5)
            tmp = gp.tile([H, HB], F32, tag="gate")
            nc.vector.scalar_tensor_tensor(tmp[:], th_r[:], 1.0, ps_nh[:, 1, :],
                                           ALU.add, ALU.mult)
            pre = gp.tile([H, HB], F32, tag="gate")
            nc.vector.tensor_add(pre[:], tmp[:], ps_nh[:, 0, :])
            n_ = gp.tile([H, HB], BF16, tag="gatez")
            nc.scalar.activation(n_[:], pre[:], AF.Tanh, bias=bin_)
            d_ = gp.tile([H, HB], BF16, tag="gatez")
            nc.vector.tensor_sub(d_[:], n_[:], hb_old[:])
            v1 = gp.tile([H, HB], BF16, tag="gatez")
            nc.vector.scalar_tensor_tensor(v1[:], th_z[:], -1.0, d_[:], ALU.add, ALU.mult)
            hb_new = hp.tile([H, HB], BF16, tag=f"hb{hf}", name=f"hbn{hf}")
            nc.vector.scalar_tensor_tensor(hb_new[:], v1[:], -0.5, hb_old[:],
                                           ALU.mult, ALU.add)
            return hb_new

        # ---------------- phase B: encoder ----------------
        for t in range(kt):
            xt = sp.tile([ENC, BS], BF16, tag="x")
            nc.sync.dma_start(out=xt[:], in_=enc_d[t])
            for hf in range(NH):
                sl = slice(hf * HB, (hf + 1) * HB)
                hb_old = hbb[hf]
                ps_rz = ppg.tile([H, 2, HB], F32, tag="grz")
                nc.tensor.matmul(ps_rz[:, 0, :], Wih_e[:, 0:H], xt[:, sl],
                                 start=True, stop=False)
                nc.tensor.matmul(ps_rz[:, 0, :], Whh_e[:, 0:H], hb_old[:],
                                 start=False, stop=True)
                nc.tensor.matmul(ps_rz[:, 1, :], Wih_e[:, H:2 * H], xt[:, sl],
                                 start=True, stop=False)
                nc.tensor.matmul(ps_rz[:, 1, :], Whh_e[:, H:2 * H], hb_old[:],
                                 start=False, stop=True)
                ps_nh = ppg.tile([H, 2, HB], F32, tag="gnh")
                nc.tensor.matmul(ps_nh[:, 0, :], Wih_e[:, 2 * H:3 * H], xt[:, sl],
                                 start=True, stop=True)
                nc.tensor.matmul(ps_nh[:, 1, :], Whh_e[:, 2 * H:3 * H], hb_old[:],
                                 start=True, stop=False)
                nc.tensor.matmul(ps_nh[:, 1, :], BR[0:1, 0:H], ones[:, sl],
                                 start=False, stop=True)
                hb_new = gru_tail(ps_rz, ps_nh, bre, bze, bine, hb_old, hf)

                # Uo[:, t, sl] = U @ h_new + Ub   (bf16)
                ps_uo = ppw.tile([H, HB], F32, tag="wh")
                nc.tensor.matmul(ps_uo[:], UW[:], hb_new[:], start=True, stop=True)
                nc.scalar.activation(Uo[:, t, sl], ps_uo[:], AF.Identity, bias=Ub)

                # encb[:, 2hf+c2, :, t] = h_new.T chunks (bf16)
                for c2 in range(NC2):
                    ptr = pptr.tile([128, 128], BF16, tag="trb")
                    nc.tensor.transpose(ptr[0:128, 0:H],
                                        hb_new[:, c2 * 128:(c2 + 1) * 128],
                                        idb[0:H, 0:H])
                    nc.scalar.copy(encb[:, NC2 * hf + c2, :, t], ptr[0:128, 0:H])
                hbb[hf] = hb_new

        # ---------------- phase C: decoder ----------------
        prevs = []
        for hf in range(NH):
            pv = hp.tile([1, HB], BF16, tag=f"pv{hf}")
            nc.sync.dma_start(out=pv[:], in_=enc_d[T - 1, 0:1, hf * HB:(hf + 1) * HB])
            prevs.append(pv)

        # wh for step 0 (subsequent steps hoist this next to the tail)
        whs = {}

        def compute_wh(hf):
            ps_wh = ppw.tile([H, HB], F32, tag="wh")
            nc.tensor.matmul(ps_wh[:], WlW[:], hbb[hf][:], start=True, stop=True)
            wh = mp.tile([H, 1, HB], BF16, tag="whb")
            nc.scalar.activation(wh[:, 0, :], ps_wh[:], AF.Identity, bias=Wlb)
            whs[hf] = wh

        for hf in range(NH):
            compute_wh(hf)

        for s in range(ks):
            dxt = sp.tile([DEC, BS], BF16, tag="dx")
            nc.sync.dma_start(out=dxt[:], in_=dec_d[s])

            scfs, attns = {}, {}
            # scores pipeline: interleave the halves' (add, tanh) chunks, with
            # each chunk's V-dot matmuls + staging copies emitted one slot
            # behind so copies never head the ACT queue ahead of a tanh
            for hf in range(NH):
                scfs[hf] = sp.tile([128, NC2, T], F32, tag=f"sco{hf}",
                                   name=f"sco{hf}")
            seq = [(hf, q) for hf in range(NH) for q in range(T // TQ)]
            xqs = {}

            def vdot_group(hf, q):
                xq = xqs[(hf, q)]
                tq = slice(q * TQ, (q + 1) * TQ)
                for c2 in range(NC2):
                    pssc = ppsc.tile([128, TQ, BBLK], F32, tag="sc")
                    for j in range(4):
                        b0 = c2 * 128 + j * BBLK
                        nc.tensor.matmul(pssc[BBLK * j:BBLK * j + 1, :, :], Vb[:],
                                         xq[:, :, b0:b0 + BBLK],
                                         start=True, stop=True,
                                         tile_position=(0, BBLK * j))
                    sstg = mp.tile([128, BBLK, TQ], F32, tag="sst")
                    nc.scalar.copy(sstg[:], pssc[:].transpose([0, 2, 1]))
                    nc.sync.dma_start(out=scfs[hf][:, c2, tq],
                                      in_=sstg[0:128:BBLK])

            for hf in range(NH):
                sl = slice(hf * HB, (hf + 1) * HB)
                for q in range(T // TQ):
                    tq = slice(q * TQ, (q + 1) * TQ)
                    xq = xqp.tile([H, TQ, HB], BF16, tag="xt")
                    nc.vector.tensor_add(xq[:], Uo[:, tq, sl],
                                         whs[hf][:].broadcast_to((H, TQ, HB)))
                    nc.scalar.activation(xq[:], xq[:], AF.Tanh)
                    xqs[(hf, q)] = xq
                    vdot_group(hf, q)

            # stage softmax (no max-subtraction; scores bounded by ||V||_1)
            # + attention weighted sum + transpose back
            for hf in range(NH):
                scf = scfs[hf]
                expo = sp.tile([128, NC2, T], F32, tag="expo")
                nc.scalar.activation(expo[:], scf[:], AF.Exp)
                sm = sp.tile([128, NC2], F32, tag="red")
                nc.vector.tensor_reduce(sm[:], expo[:], axis=AX.X, op=ALU.add)
                inv = sp.tile([128, NC2], F32, tag="red2")
                nc.vector.reciprocal(inv[:], sm[:])
                ab = sp.tile([128, NC2, 1, T], BF16, tag="ab")
                for c2 in range(NC2):
                    nc.vector.tensor_scalar_mul(ab[:, c2, 0, :], expo[:, c2, :],
                                                inv[:, c2:c2 + 1])
                attn_h = mp.tile([H, HB], BF16, tag="ah")
                TH = T // 2
                for c2 in range(NC2):
                    # alpha-weighted products, then in-place bf16 fold tree
                    # (TT adds run 2x; TensorReduce only 1x) down to 7 t-slots
                    P1 = pbp.tile([128, H, TH], BF16, tag="P")
                    nc.vector.tensor_mul(
                        P1[:], encb[:, NC2 * hf + c2, :, 0:TH],
                        ab[:, c2, :, 0:TH].broadcast_to((128, H, TH)))
                    P2 = pbp.tile([128, H, TH], BF16, tag="P")
                    nc.vector.tensor_mul(
                        P2[:], encb[:, NC2 * hf + c2, :, TH:T],
                        ab[:, c2, :, TH:T].broadcast_to((128, H, TH)))
                    nc.vector.tensor_add(P1[:], P1[:], P2[:])
                    nc.vector.tensor_add(P1[:, :, 0:14], P1[:, :, 0:14], P1[:, :, 14:28])
                    nc.vector.tensor_add(P1[:, :, 0:7], P1[:, :, 0:7], P1[:, :, 7:14])
                    attnc = sp.tile([128, H], BF16, tag="attnc")
                    with nc.allow_low_precision(reason="bf16 attn t-reduce"):
                        nc.vector.tensor_reduce(attnc[:], P1[:, :, 0:7], axis=AX.X,
                                                op=ALU.add)
                    ptr = pptr.tile([128, 128], BF16, tag="trb")
                    nc.tensor.transpose(ptr[0:H, 0:128], attnc[:], idb[:])
                    nc.scalar.copy(attn_h[:, c2 * 128:(c2 + 1) * 128], ptr[0:H, 0:128])
                attns[hf] = attn_h

            # stage gates + tail + out per half
            for hf in range(NH):
                sl = slice(hf * HB, (hf + 1) * HB)
                hb_old = hbb[hf]
                attn_h = attns[hf]
                prev = prevs[hf]
                ps_rz = ppg.tile([H, 2, HB], F32, tag="grz")
                for gi, g0 in ((0, 0), (1, H)):
                    nc.tensor.matmul(ps_rz[:, gi, :], WihP[:, g0:g0 + H], prev[:],
                                     start=True, stop=False)
                    nc.tensor.matmul(ps_rz[:, gi, :], WihX[:, g0:g0 + H], dxt[:, sl],
                                     start=False, stop=False)
                    nc.tensor.matmul(ps_rz[:, gi, :], WihA[:, g0:g0 + H], attn_h[:],
                                     start=False, stop=False)
                    nc.tensor.matmul(ps_rz[:, gi, :], Whh_dd[:, g0:g0 + H], hb_old[:],
                                     start=False, stop=True)
                ps_nh = ppg.tile([H, 2, HB], F32, tag="gnh")
                g0 = 2 * H
                nc.tensor.matmul(ps_nh[:, 0, :], WihP[:, g0:g0 + H], prev[:],
                                 start=True, stop=False)
                nc.tensor.matmul(ps_nh[:, 0, :], WihX[:, g0:g0 + H], dxt[:, sl],
                                 start=False, stop=False)
                nc.tensor.matmul(ps_nh[:, 0, :], WihA[:, g0:g0 + H], attn_h[:],
                                 start=False, stop=True)
                nc.tensor.matmul(ps_nh[:, 1, :], Whh_dd[:, g0:g0 + H], hb_old[:],
                                 start=True, stop=False)
                nc.tensor.matmul(ps_nh[:, 1, :], BR[0:1, H:2 * H], ones[:, sl],
                                 start=False, stop=True)
                hb_new = gru_tail(ps_rz, ps_nh, brd, bzd, bind, hb_old, hf)

                # out_s = h2o @ h_new + b  -> DRAM, also feeds prev
                ps_o = pptr.tile([1, HB], F32, tag="osc")
                nc.tensor.matmul(ps_o[:], h2oWb[:], hb_new[:], start=True, stop=True)
                pv = hp.tile([1, HB], BF16, tag=f"pv{hf}")
                nc.scalar.activation(pv[:], ps_o[:], AF.Identity, bias=h2ob[:, 0:1])
                nc.sync.dma_start(out=out_d[s, sl], in_=pv[:])
                prevs[hf] = pv
                hbb[hf] = hb_new
                if s < ks - 1:
                    compute_wh(hf)
    _split_excess_waits(nc)
    return nc


def _host_inputs(inputs):
    import ml_dtypes
    f = lambda a: np.ascontiguousarray(a, dtype=np.float32)
    g = lambda a: np.ascontiguousarray(np.asarray(a, dtype=np.float32),
                                       dtype=ml_dtypes.bfloat16)
    Whh_e = np.asarray(inputs["enc_Whh"]).T.copy()   # [H, 3H]
    Whh_d = np.asarray(inputs["dec_Whh"]).T.copy()
    Whh_e[:, 2 * H:3 * H] *= 0.5
    Whh_d[:, 2 * H:3 * H] *= 0.5
    shared = {
        "W1": g(inputs["s2h_W1"].T), "W2": g(inputs["s2h_W2"].T),
        "b1": f(np.asarray(inputs["s2h_b1"]).reshape(96, 1)),
        "Wih_e": g(inputs["enc_Wih"].T), "Whh_e": g(Whh_e),
        "WihP_d": g(inputs["dec_Wih"][:, 0:1].T),
        "WihX_d": g(inputs["dec_Wih"][:, 1:1 + DEC].T),
        "WihA_d": g(inputs["dec_Wih"][:, 1 + DEC:].T),
        "Whh_d": g(Whh_d),
        "UW": g(inputs["U_W"].T), "WlW": g(inputs["Wl_W"].T),
        "h2ob": f(np.asarray(inputs["h2o_b"]).reshape(1, 1)),
        "ident": f(np.eye(128)),
    }
    BI = np.zeros((H, 16), dtype=np.float32)
    ebih, ebhh = np.asarray(inputs["enc_bih"]), np.asarray(inputs["enc_bhh"])
    dbih, dbhh = np.asarray(inputs["dec_bih"]), np.asarray(inputs["dec_bhh"])
    BI[:, 0] = (ebih[0:H] + ebhh[0:H]) / 2
    BI[:, 1] = (ebih[H:2 * H] + ebhh[H:2 * H]) / 2
    BI[:, 2] = ebih[2 * H:3 * H]
    BI[:, 4] = (dbih[0:H] + dbhh[0:H]) / 2
    BI[:, 5] = (dbih[H:2 * H] + dbhh[H:2 * H]) / 2
    BI[:, 6] = dbih[2 * H:3 * H]
    BI[:, 8] = np.asarray(inputs["U_b"])
    BI[:, 9] = np.asarray(inputs["Wl_b"])
    BI[:, 10] = np.asarray(inputs["V_W"]).reshape(-1)
    BI[:, 11] = np.asarray(inputs["h2o_W"]).reshape(-1)
    BI[:, 12] = np.asarray(inputs["s2h_b2"])
    shared["BIAS"] = f(BI)
    BR = np.zeros((1, 2 * H), dtype=np.float32)
    BR[0, 0:H] = ebhh[2 * H:3 * H] / 2
    BR[0, H:2 * H] = dbhh[2 * H:3 * H] / 2
    shared["BR"] = g(BR)

    enc = np.asarray(inputs["encoder_data"])   # [T, B, ENC]
    dec = np.asarray(inputs["decoder_data"])   # [S, B, DEC]
    ann = np.asarray(inputs["ann_data"])       # [B, ANN]
    maps = []
    for i in range(NCORES):
        sl = slice(i * BS, (i + 1) * BS)
        m = dict(shared)
        m["enc"] = g(enc[:, sl, :].transpose(0, 2, 1))
        m["dec"] = g(dec[:, sl, :].transpose(0, 2, 1))
        m["ann"] = g(ann[sl, :].T)
        maps.append(m)
    return maps


def kernel(**inputs) -> np.ndarray:
    from concourse.bass_utils import run_bass_kernel_spmd
    if "nc" not in _CACHE:
        _CACHE["nc"] = _build_program()
    nc = _CACHE["nc"]
    maps = _host_inputs(inputs)
    import os
    kw = {}
    if os.environ.get("KERNEL_TRACE") == "1":
        kw = dict(trace=True, trace_cores=[0])
    res = run_bass_kernel_spmd(nc, maps, list(range(NCORES)), **kw)
    _CACHE["last_res"] = res
    outs = [np.asarray(res.results[i]["out"], dtype=np.float32) for i in range(NCORES)]
    full = np.stack(outs, axis=1)              # [S, 8, 512]
    return full.reshape(S, B, 1)


if __name__ == "__main__":
    rng = np.random.default_rng(0)
    fake = {}
    fake["ann_data"] = rng.standard_normal((B, ANN), dtype=np.float32)
    fake["encoder_data"] = rng.standard_normal((T, B, ENC), dtype=np.float32)
    fake["decoder_data"] = rng.standard_normal((S, B, DEC), dtype=np.float32)
    p = lambda *s: (rng.standard_normal(s, dtype=np.float32) * 0.08)
    fake.update({
        "s2h_W1": p(96, ANN), "s2h_b1": p(96), "s2h_W2": p(H, 96), "s2h_b2": p(H),
        "enc_Wih": p(3 * H, ENC), "enc_Whh": p(3 * H, H),
        "enc_bih": p(3 * H), "enc_bhh": p(3 * H),
        "dec_Wih": p(3 * H, 1 + DEC + H), "dec_Whh": p(3 * H, H),
        "dec_bih": p(3 * H), "dec_bhh": p(3 * H),
        "U_W": p(H, H), "U_b": p(H), "Wl_W": p(H, H), "Wl_b": p(H),
        "V_W": p(1, H), "V_b": p(1), "h2o_W": p(1, H), "h2o_b": p(1),
    })
    out = kernel(**fake)
    print("out", out.shape, out.dtype, float(np.abs(out).mean()))
